# revision 61
# baseline (speedup 1.0000x reference)
"""Causal self-attention (GPT-style block) on 8 Trainium2 NeuronCores.

Problem: x[4,2048,1024] -> qkv = x@W_attn+b ; 16-head causal attention
(head_dim 64) ; out = y@W_proj+b_proj.

Sharding: tensor-parallel over heads. Core c owns heads {2c, 2c+1}:
  - computes q^T/k^T (feature-major) and v (key-major, natural layout)
    for its heads over the full batch via matmuls against a host-
    pretransposed x^T (bf16). v-natural comes from using the x^T tile as
    the stationary operand (out partitions = tokens), so no DMA
    transposes / DRAM round trip are needed.
  - runs causal attention for its 8 (batch, head) pairs entirely in SBUF
    (S^T layout: scores tile [128 j, 512 i]; j-tiles processed in pairs
    sharing one [128, 1024] PSUM tile so exp runs as one ScalarE
    activation per pair; causal mask via affine_select on the diagonal
    band only - fully masked leading columns are simply never read,
    because the PV matmul accumulates into yps[:, i0:] per tile; PV
    appends a ones-column to v producing y_raw^T and the softmax
    denominator in one PSUM tile),
  - per batch, a striped AllToAll (launched as soon as that batch's
    attention finishes, overlapping the next batch's compute) swaps the
    head dim for the row dim; the output projection for the core's
    4x256 rows runs after attention, pipelined with the last collective.

Bias handling: b_k drops out of softmax exactly (constant per query row);
b_v is folded into b_proj on the host (attention rows sum to 1); only
b_q is applied on device.

Numerics: bf16 operands with fp32 PSUM accumulation everywhere; softmax
skips the max-subtraction (scores are O(1) by construction; exp stays
finite) which matches the reference to ~1e-5 in fp32.
"""

import numpy as np
import ml_dtypes
from collections import deque
from contextlib import ExitStack

import concourse.bass as bass
import concourse.tile as tile
from concourse import bacc, mybir
from concourse.bass_utils import run_bass_kernel_spmd
from concourse.tile_rust import add_dep_helper

F32 = mybir.dt.float32
BF16 = mybir.dt.bfloat16
F8 = mybir.dt.float8e4
AF = mybir.ActivationFunctionType

N_CORES = 8
B, T, C, H = 4, 2048, 1024, 16
HD = C // H            # 64 head dim
HPC = H // N_CORES     # 2 heads per core
FPC = HPC * HD         # 128 features per core
BT = B * T             # 8192 rows
TCHUNK = 512           # t chunk in qkv phase
NT_CHUNKS = BT // TCHUNK
QB = 512               # query block
NQB = T // QB          # 4 per batch
JTN = T // 128         # 16 j-tiles per batch
ROWS = BT // N_CORES   # 1024 rows per core after AllToAll
KC = C // 128          # 8 contraction tiles over C
VW = 80                # [V | 1 | pad] row unit in vsb
STRIPE = ROWS // B     # 256 rows per (core, batch): striped AllToAll
SCALE = 1.0 / np.sqrt(HD)

# If the overlapped (per-batch, concurrent-with-compute) AllToAll turns
# out to corrupt data on HW, set False to emit all collectives after the
# attention loop (still chunked and pipelined with the out projection).
OVERLAP_A2A = True

# AllToAll chunking: chunk i ships CHUNK_ROWS[i] rows per (core, dest).
# Chunks 0-2 are batches 0-2 (dest r takes that batch's contiguous rows
# r*256..+256 -> one contiguous stage DMA per dest+head, clustered per
# batch: scattered small staging DMAs during attention deepen HW power
# throttling). Batch 3 fires as qb0+qb1 (after qb1), qb2, and qb3 so the
# collective gating the tail is only 128KB.
CHUNK_ROWS = (256, 256, 256, 128, 64, 64)
CHUNK_R0 = (0, 256, 512, 768, 896, 960)      # device-row base per chunk
CHUNK_GBASE = (0, 2048, 4096, 6144, 7168, 7680)  # global-row base per chunk
N_CHUNKS = len(CHUNK_ROWS)

# Interleave qkv/proj matmuls between attention pairs. Keeps every engine
# busy, but the extra concurrency deepens the chip's activity-based power
# throttle and measured net-slower on HW; False = phase-separated.
INTERLEAVE = False

LAST_RESULTS = None    # test.py reads exec_time_ns off this


def build_program(nc):
    xT = nc.dram_tensor("xT", [C, BT], BF16, kind="ExternalInput").ap()
    wq = nc.dram_tensor("wq", [C, FPC], BF16, kind="ExternalInput").ap()
    wk = nc.dram_tensor("wk", [C, FPC], BF16, kind="ExternalInput").ap()
    wv = nc.dram_tensor("wv", [C, FPC], BF16, kind="ExternalInput").ap()
    bqkv = nc.dram_tensor("bqkv", [3, FPC], F32, kind="ExternalInput").ap()
    wp = nc.dram_tensor("wp", [C, C], BF16, kind="ExternalInput").ap()
    bp = nc.dram_tensor("bp", [C], F32, kind="ExternalInput").ap()
    msk = nc.dram_tensor("msk", [128, 128], BF16, kind="ExternalInput").ap()
    out = nc.dram_tensor("out", [ROWS, C], F32, kind="ExternalOutput").ap()
    # round-robin 128-row blocks: global block m (of 64) -> core m%8, so a
    # collective chunk covering any 8*L consecutive blocks addresses all 8
    # destinations. Chunks fire after b1, b2, (b3,qb1), (b3,qb3) with
    # L = 4, 2, 1, 1 blocks per destination.
    cc_in = [
        nc.dram_tensor(f"cc_in{i}", [N_CORES, FPC, R], BF16, kind="Internal").ap()
        for i, R in enumerate(CHUNK_ROWS)
    ]
    cc_out = [
        nc.dram_tensor(f"cc_out{i}", [N_CORES, FPC, R], BF16, kind="Internal").ap()
        for i, R in enumerate(CHUNK_ROWS)
    ]

    with tile.TileContext(nc) as tc:
        with ExitStack() as ctx:
            emit(ctx, tc, xT, wq, wk, wv, bqkv, wp, bp, msk, out, cc_in, cc_out)
    return nc


def emit(ctx, tc, xT, wq, wk, wv, bqkv, wp, bp, msk, out, cc_in, cc_out):
    nc = tc.nc
    res = ctx.enter_context(tc.tile_pool(name="resident", bufs=1))

    # ---------- resident SBUF ----------
    qT = res.tile([128, BT], BF16)
    kT = res.tile([128, BT], BF16)
    vsb = res.tile([128, B * JTN, HPC, VW], BF16)     # [V | 1 | pad] per j-tile/head
    wq_sb = res.tile([128, KC, FPC], BF16)
    wk_sb = res.tile([128, KC, FPC], BF16)
    wv_sb = res.tile([128, KC, FPC], BF16)
    b_sb = res.tile([128, 3], F32)
    wp_sb = res.tile([128, KC, C], BF16)
    bp_sb = res.tile([128, C], F32)
    yT0 = res.tile([64, BT], BF16)
    yT1 = res.tile([64, BT], BF16)
    msk_sb = res.tile([128, 128], BF16)
    yfull = [res.tile([128, KC, R], BF16, name=f"yf{i}")
             for i, R in enumerate(CHUNK_ROWS)]

    # ---------- constant/weight loads (wp/bp deferred to phase 3) ----------
    nc.sync.dma_start(wq_sb[:], wq.rearrange("(a p) m -> p a m", p=128))
    nc.sync.dma_start(wk_sb[:], wk.rearrange("(a p) m -> p a m", p=128))
    nc.sync.dma_start(wv_sb[:], wv.rearrange("(a p) m -> p a m", p=128))
    nc.sync.dma_start(b_sb[:], bqkv.rearrange("b p -> p b"))
    nc.sync.dma_start(msk_sb[:], msk)
    nc.vector.memset(vsb[:, :, :, HD : HD + 1], 1.0)

    # ---------- pools ----------
    # PSUM is 8 banks. INTERLEAVE shares one 2-bank tag across qkv+proj so
    # everything coexists; phase-separated mode scopes pools per phase for
    # deeper per-phase buffering (qkv 4 banks -> scores 6+yps 2 -> proj 4).
    xpool = ctx.enter_context(tc.tile_pool(name="xt", bufs=3))
    ptpool = ctx.enter_context(tc.tile_pool(name="pt", bufs=3))
    npool = ctx.enter_context(tc.tile_pool(name="norm", bufs=3))
    ospool = ctx.enter_context(tc.tile_pool(name="osb", bufs=3))
    ypool = ctx.enter_context(tc.tile_pool(name="yps", bufs=2, space="PSUM"))
    pools = {}
    ph1 = ExitStack()
    if INTERLEAVE:
        qkvps = ctx.enter_context(tc.tile_pool(name="qkvps", bufs=2, space="PSUM"))
        spool = ctx.enter_context(tc.tile_pool(name="sps", bufs=2, space="PSUM"))
        pools["qkv"] = pools["v"] = pools["proj"] = qkvps
        pools["qtag"] = pools["vtag"] = pools["ptag0"] = pools["ptag1"] = "qkvps"
    else:
        pools["qkv"] = ph1.enter_context(tc.tile_pool(name="qkvps", bufs=2, space="PSUM"))
        pools["v"] = ph1.enter_context(tc.tile_pool(name="vps", bufs=2, space="PSUM"))
        pools["qtag"], pools["vtag"] = "qkvps", "vps"
        pools["ptag0"], pools["ptag1"] = "ops0", "ops1"
    xT_t = xT.rearrange("(a p) t -> p a t", p=128)
    NSUB = TCHUNK // 128
    yT = (yT0, yT1)

    # ---------- filler queue: qkv/proj PE work interleaved between pairs ----
    # Attention is ScalarE(exp)-paced; the PE would idle ~0.5us per pair.
    # Instead of monolithic phases, qkv-projection and output-projection
    # matmuls are queued as small closures and popped between attention
    # pairs, keeping both engines saturated from ~25us onward.
    filler = deque()

    def pop_filler(n):
        done = 0
        while done < n and filler:
            f = filler.popleft()
            if callable(f):
                f()
                done += 1

    def drain_filler(marker):
        while marker in filler:
            f = filler.popleft()
            if f is not marker:
                f()

    def drain_all_filler():
        while filler:
            f = filler.popleft()
            if callable(f):
                f()

    chunk_marker = {}

    def reg_chunk(tci):
        t0 = tci * TCHUNK
        xt = xpool.tile([128, KC, TCHUNK], BF16, tag="xt", name=f"xt{tci}")
        # split the 1 MiB chunk load across DMA queues, issued now (one
        # qb-block of lead time before the matmul closures drain); chunk 0
        # gates the first matmul, so split it per a-tile
        nspl = 8 if tci == 0 else 4
        w = KC // nspl
        for spl in range(nspl):
            nc.sync.dma_start(
                xt[:, w * spl : w * (spl + 1), :],
                xT_t[:, w * spl : w * (spl + 1), t0 : t0 + TCHUNK],
            )
        cell = {}

        def qk_mms(bi, w_sb, a0):
            def go():
                if a0 == 0:
                    cell[bi] = pools["qkv"].tile([128, NSUB, HPC, HD], F32,
                                          tag=pools["qtag"], name=f"qk{tci}_{bi}")
                for a in (a0, a0 + 1):
                    nc.tensor.matmul(
                        cell[bi][:], lhsT=w_sb[:, a, :], rhs=xt[:, a, :],
                        start=(a == 0), stop=(a == KC - 1),
                    )
            return go

        def qk_evict(bi, dst):
            def go():
                # evictions on DVE: keeps ScalarE exp-only.  b_q applied;
                # b_k is softmax-invariant (constant per query row), dropped.
                if bi == 0:
                    nc.vector.tensor_scalar_add(
                        dst[:, t0 : t0 + TCHUNK], cell[bi][:], b_sb[:, 0:1]
                    )
                else:
                    nc.vector.tensor_copy(dst[:, t0 : t0 + TCHUNK], cell[bi][:])
            return go

        def v_mms(tt, a0):
            def go():
                # v in natural layout: out partitions = tokens, lhsT = x^T
                if tt == 0 and a0 == 0:
                    cell[2] = pools["v"].tile([128, NSUB, HPC, HD], F32,
                                         tag=pools["vtag"], name=f"v{tci}")
                for a in range(a0, a0 + 4):
                    nc.tensor.matmul(
                        cell[2][:, tt, :, :],
                        lhsT=xt[:, a, tt * 128 : (tt + 1) * 128],
                        rhs=wv_sb[:, a, :],
                        start=(a == 0), stop=(a == KC - 1),
                    )
            return go

        def v_evict():
            # b_v is folded into b_proj on the host (attn rows sum to 1)
            g0 = NSUB * tci
            nc.vector.tensor_copy(vsb[:, g0 : g0 + NSUB, :, 0:HD], cell[2][:])

        for bi, w_sb, dst in ((0, wq_sb, qT), (1, wk_sb, kT)):
            for a0 in range(0, KC, 2):
                filler.append(qk_mms(bi, w_sb, a0))
            filler.append(qk_evict(bi, dst))
        for tt in range(NSUB):
            for a0 in range(0, KC, 4):
                filler.append(v_mms(tt, a0))
        filler.append(v_evict)
        m = object()
        chunk_marker[tci] = m
        filler.append(m)

    def reg_proj(i):
        # readback emitted here (not at fire time): a queued DMA waiting on
        # a collective would block later staging DMAs behind it in FIFO
        nc.sync.dma_start(yfull[i][:], cc_out[i].rearrange("r p t -> p r t"))
        for tt in range(CHUNK_ROWS[i] // 128 or 1):
            rows = min(128, CHUNK_ROWS[i])
            cell = {}

            def p_mms(tt, cell, a0, rows):
                def go():
                    if a0 == 0:
                        cell[0] = pools["proj"].tile([128, 512], F32,
                                             tag=pools["ptag0"], name=f"op{i}_{tt}_0")
                        cell[1] = pools["proj"].tile([128, 512], F32,
                                             tag=pools["ptag1"], name=f"op{i}_{tt}_1")
                    for a in (a0, a0 + 1):
                        lhsT = yfull[i][:, a, tt * 128 : tt * 128 + rows]
                        nc.tensor.matmul(cell[0][0:rows, 0:512], lhsT=lhsT,
                                         rhs=wp_sb[:, a, 0:512],
                                         start=(a == 0), stop=(a == KC - 1))
                        nc.tensor.matmul(cell[1][0:rows, 0:512], lhsT=lhsT,
                                         rhs=wp_sb[:, a, 512:C],
                                         start=(a == 0), stop=(a == KC - 1))
                return go

            def p_evict(tt, cell, rows):
                def go():
                    osb = ospool.tile([128, C], F32, tag="osb",
                                      name=f"osb{i}_{tt}")
                    nc.vector.tensor_add(osb[0:rows, 0:512],
                                         cell[0][0:rows, 0:512],
                                         bp_sb[0:rows, 0:512])
                    nc.vector.tensor_add(osb[0:rows, 512:C],
                                         cell[1][0:rows, 0:512],
                                         bp_sb[0:rows, 512:C])
                    r0 = CHUNK_R0[i] + tt * 128
                    nc.sync.dma_start(out[r0 : r0 + rows, :],
                                      osb[0:rows, :])
                return go

            for a0 in range(0, KC, 2):
                filler.append(p_mms(tt, cell, a0, rows))
            filler.append(p_evict(tt, cell, rows))

    def stage_batch(b):
        # chunk b: dest r gets the batch's contiguous rows r*256..+256
        for r in range(N_CORES):
            c0 = b * T + r * 2 * 128
            nc.sync.dma_start(cc_in[b][r, 0:HD, :], yT0[:, c0 : c0 + 256])
            nc.sync.dma_start(cc_in[b][r, HD:FPC, :], yT1[:, c0 : c0 + 256])

    def stage_b3_qb(qb):
        # chunk 3 (qb0+qb1, dest r <- b3 row block 128r) or chunk 4/5
        # (qb2/qb3, dest r <- 64-row block within the qb)
        if qb < 2:
            for u in range(4):
                r = 4 * qb + u
                c0 = 3 * T + r * 128
                nc.sync.dma_start(cc_in[3][r, 0:HD, :], yT0[:, c0 : c0 + 128])
                nc.sync.dma_start(cc_in[3][r, HD:FPC, :], yT1[:, c0 : c0 + 128])
        else:
            i = qb + 2
            for r in range(N_CORES):
                c0 = 3 * T + qb * QB + r * 64
                nc.sync.dma_start(cc_in[i][r, 0:HD, :], yT0[:, c0 : c0 + 64])
                nc.sync.dma_start(cc_in[i][r, HD:FPC, :], yT1[:, c0 : c0 + 64])

    last_aff = [None]

    def fire_a2a(i, after_attn=True):
        cc = nc.gpsimd.collective_compute(
            "AllToAll", mybir.AluOpType.bypass,
            ins=[cc_in[i]], outs=[cc_out[i]],
            replica_groups=[list(range(N_CORES))],
        )
        # pin the trigger behind already-emitted attention gpsimd work:
        # the scheduler otherwise hoists its staging-semaphore waits to
        # the front of the gpsimd queue, stalling affine_selects (and with
        # them the whole exp->PV pipeline) for 10-17us per batch boundary
        if after_attn and last_aff[0] is not None:
            add_dep_helper(cc.ins, last_aff[0].ins, True,
                           "defer A2A trigger behind attention")

    # ---------- interleaved qkv + attention ----------
    # Chunk tci=4b+qb+1 registers (DMA now, matmul closures queued) at the
    # start of qb block (b,qb) and is force-drained by that block's end --
    # exactly when block (b,qb+1) first needs its q/k/v.
    reg_chunk(0)
    drain_filler(chunk_marker[0])
    if not INTERLEAVE:
        for tci in range(1, NT_CHUNKS):
            reg_chunk(tci)
        drain_all_filler()
        ph1.close()  # free qkv PSUM banks for the attention pools
        ph23 = ExitStack()
        spool = ph23.enter_context(tc.tile_pool(name="sps", bufs=3, space="PSUM"))
    for b in range(B):
        if b == 1:
            nc.sync.dma_start(wp_sb[:], wp.rearrange("(a p) e -> p a e", p=128))
            bp_bcast = bass.AP(tensor=bp.tensor, offset=bp.offset,
                               ap=[[0, 128], [1, C]])
            nc.sync.dma_start(bp_sb[:], bp_bcast)
        for qb in range(NQB):
            nxt = 4 * b + qb + 1
            if INTERLEAVE and nxt < NT_CHUNKS:
                reg_chunk(nxt)
            if OVERLAP_A2A:
                # output projection for landed A2A chunks becomes filler too
                if b == 2 and qb == 0:
                    reg_proj(0)
                elif b == 3 and qb == 0:
                    reg_proj(1)
                elif b == 3 and qb == 3:
                    # after fire(2) at qb2-end: a readback emitted before
                    # its collective would read stale DRAM (NaNs)
                    reg_proj(2)
                    reg_proj(3)
                    # chunk 4 (qb2's rows) staged last block; firing here
                    # overlaps its collective with qb3's attention
                    fire_a2a(4)
            q0g = b * T + qb * QB
            njt = (qb + 1) * (QB // 128)
            yps = [
                ypool.tile([HD + 1, QB], F32, tag="yps", name=f"yp{b}_{qb}_{h}")
                for h in range(HPC)
            ]
            for pj in range(njt // 2):
                i0s = []
                sps = [spool.tile([128, 2 * QB], F32, tag="sps", name=f"sp{b}_{qb}_{pj}_{h}")
                       for h in range(HPC)]
                pts = [ptpool.tile([128, 2 * QB], BF16, tag="pt", name=f"pt{b}_{qb}_{pj}_{h}")
                       for h in range(HPC)]
                for jj in range(2):
                    j = 2 * pj + jj
                    j0g = b * T + j * 128
                    i0 = max(0, j * 128 - qb * QB)  # first unmasked query col
                    i0s.append(i0)
                    for h in range(HPC):
                        hs = slice(h * HD, (h + 1) * HD)
                        nc.tensor.matmul(
                            sps[h][:, jj * QB + i0 : (jj + 1) * QB],
                            lhsT=kT[hs, j0g : j0g + 128],
                            rhs=qT[hs, q0g + i0 : q0g + QB], start=True, stop=True,
                        )
                for h in range(HPC):
                    # one exp per j-tile pair; [512+i0s[0] : 512+i0s[1]) holds
                    # exp(garbage) but is never read (PV skips masked cols)
                    nc.scalar.activation(
                        pts[h][:, i0s[0] : 2 * QB], sps[h][:, i0s[0] : 2 * QB],
                        AF.Exp, scale=float(SCALE),
                    )
                    for jj in range(2):
                        j = 2 * pj + jj
                        if j * 128 + 127 > qb * QB:
                            # boundary tile: keep where j <= i on the 128-wide band
                            i0 = i0s[jj]
                            band = slice(jj * QB + i0, jj * QB + i0 + 128)
                            last_aff[0] = nc.gpsimd.affine_select(
                                pts[h][:, band], pts[h][:, band],
                                pattern=[[1, 128]], base=0, channel_multiplier=-1,
                                compare_op=mybir.AluOpType.is_ge, fill=0.0,
                            )
                for jj in range(2):
                    j = 2 * pj + jj
                    i0 = i0s[jj]
                    for h in range(HPC):
                        nc.tensor.matmul(
                            yps[h][:, i0:QB],
                            lhsT=vsb[:, b * JTN + j, h, 0 : HD + 1],
                            rhs=pts[h][:, jj * QB + i0 : (jj + 1) * QB],
                            start=(j == 0), stop=(j == njt - 1),
                        )
                if INTERLEAVE:
                    pop_filler(4)
            # softmax normalization: row HD of yp is the denominator.
            # One fast PSUM->SBUF copy releases the yp bank; the recip /
            # broadcast / scale chain then runs off SBUF.
            for h in range(HPC):
                ln = npool.tile([1, QB], F32, tag="ln")
                nc.vector.tensor_copy(ln[:], yps[h][HD : HD + 1, :])
                yraw = npool.tile([HD, QB], F32, tag="yraw")
                nc.vector.tensor_copy(yraw[:], yps[h][0:HD, :])
                rn = npool.tile([1, QB], F32, tag="rn")
                nc.vector.reciprocal_approx_fast(rn[:], ln[:])
                rb = npool.tile([HD, QB], F32, tag="rb")
                nc.gpsimd.partition_broadcast(rb[:], rn[:], channels=HD)
                nc.vector.tensor_mul(yT[h][:, q0g : q0g + QB], yraw[:], rb[:])
            if INTERLEAVE and nxt < NT_CHUNKS:
                drain_filler(chunk_marker[nxt])
            if OVERLAP_A2A:
                # stage at the producing qb's end; defer each fire by one
                # qb-block so the trigger's semaphore waits (which block the
                # gpsimd queue, and with it the next affine_selects) resolve
                # before the trigger reaches the queue head.
                if qb == 2 and b > 0:
                    # three qb-blocks after staging: the staging DMAs
                    # (which complete ~20us after emission) are done well
                    # before the trigger is scheduled, so its semaphore
                    # waits never block the gpsimd queue
                    fire_a2a(b - 1)
                if b < 3 and qb == 3:
                    stage_batch(b)
                elif b == 3:
                    if qb == 2:
                        fire_a2a(3)
                    stage_b3_qb(qb)
                    if qb == 3:
                        fire_a2a(5, after_attn=False)

    if not INTERLEAVE:
        ph23.close()  # free the scores banks; proj gets its own 4 banks
        pools["proj"] = ctx.enter_context(
            tc.tile_pool(name="ops", bufs=2, space="PSUM"))
    drain_all_filler()

    if not OVERLAP_A2A:
        for b in range(3):
            stage_batch(b)
        for qb in range(NQB):
            stage_b3_qb(qb)
        for i in range(N_CHUNKS):
            fire_a2a(i)
        for i in range(4):
            reg_proj(i)

    # tail: projection for the last two (small) A2A chunks
    reg_proj(4)
    reg_proj(5)
    drain_all_filler()


_COMPILED_NC = None


def _get_nc():
    global _COMPILED_NC
    if _COMPILED_NC is None:
        nc = bacc.Bacc("TRN2", target_bir_lowering=False, debug=False,
                       num_devices=N_CORES)
        build_program(nc)
        nc.compile()
        _COMPILED_NC = nc
    return _COMPILED_NC


def kernel(x, W_attn, b_attn, W_proj, b_proj):
    global LAST_RESULTS
    nc = _get_nc()

    bf = ml_dtypes.bfloat16
    xT_np = np.ascontiguousarray(
        np.asarray(x, np.float32).reshape(BT, C).T
    ).astype(bf)
    W_attn = np.asarray(W_attn, np.float32)
    b_attn = np.asarray(b_attn, np.float32)
    W_proj = np.asarray(W_proj, np.float32)
    wp_np = W_proj.astype(bf)
    # b_v folds into b_proj: attention rows sum to 1, so y += b_v exactly
    bp_np = (np.asarray(b_proj, np.float32) + b_attn[2 * C :] @ W_proj).astype(np.float32)
    # causal band mask in S^T layout: keep where query col >= key row
    msk_np = np.triu(np.ones((128, 128), np.float32)).astype(bf)

    in_maps = []
    for c in range(N_CORES):
        s = slice(c * FPC, (c + 1) * FPC)
        in_maps.append({
            "xT": xT_np,
            "wq": np.ascontiguousarray(W_attn[:, s]).astype(bf),
            "wk": np.ascontiguousarray(W_attn[:, C:2 * C][:, s]).astype(bf),
            "wv": np.ascontiguousarray(W_attn[:, 2 * C:][:, s]).astype(bf),
            "bqkv": np.ascontiguousarray(
                np.stack([b_attn[s], b_attn[C:2 * C][s], b_attn[2 * C:][s]])
            ).astype(np.float32),
            "wp": wp_np,
            "bp": bp_np,
            "msk": msk_np,
        })

    res = run_bass_kernel_spmd(nc, in_maps, core_ids=list(range(N_CORES)))
    LAST_RESULTS = res
    # gather: chunk i of core c covers global rows GBASE[i] + c*ROWS[i] ..
    arr = np.stack([res.results[c]["out"] for c in range(N_CORES)], axis=0)
    full = np.empty((BT, C), np.float32)
    for i in range(N_CHUNKS):
        g, r0, rows = CHUNK_GBASE[i], CHUNK_R0[i], CHUNK_ROWS[i]
        for c in range(N_CORES):
            full[g + c * rows : g + (c + 1) * rows] = arr[c, r0 : r0 + rows]
    return full.reshape(B, T, C)


# revision 62
# speedup vs baseline: 1.0275x; 1.0275x over previous
"""Causal self-attention (GPT-style block) on 8 Trainium2 NeuronCores.

Problem: x[4,2048,1024] -> qkv = x@W_attn+b ; 16-head causal attention
(head_dim 64) ; out = y@W_proj+b_proj.

Sharding: tensor-parallel over heads. Core c owns heads {2c, 2c+1}:
  - computes q^T/k^T (feature-major) and v (key-major, natural layout)
    for its heads over the full batch via matmuls against a host-
    pretransposed x^T (bf16). v-natural comes from using the x^T tile as
    the stationary operand (out partitions = tokens), so no DMA
    transposes / DRAM round trip are needed.
  - runs causal attention for its 8 (batch, head) pairs entirely in SBUF
    (S^T layout: scores tile [128 j, 512 i]; j-tiles processed in pairs
    sharing one [128, 1024] PSUM tile so exp runs as one ScalarE
    activation per pair; causal mask via affine_select on the diagonal
    band only - fully masked leading columns are simply never read,
    because the PV matmul accumulates into yps[:, i0:] per tile; PV
    appends a ones-column to v producing y_raw^T and the softmax
    denominator in one PSUM tile),
  - per batch, a striped AllToAll (launched as soon as that batch's
    attention finishes, overlapping the next batch's compute) swaps the
    head dim for the row dim; the output projection for the core's
    4x256 rows runs after attention, pipelined with the last collective.

Bias handling: b_k drops out of softmax exactly (constant per query row);
b_v is folded into b_proj on the host (attention rows sum to 1); only
b_q is applied on device.

Numerics: bf16 operands with fp32 PSUM accumulation everywhere; softmax
skips the max-subtraction (scores are O(1) by construction; exp stays
finite) which matches the reference to ~1e-5 in fp32.
"""

import numpy as np
import ml_dtypes
from collections import deque
from contextlib import ExitStack

import concourse.bass as bass
import concourse.tile as tile
from concourse import bacc, mybir
from concourse.bass_utils import run_bass_kernel_spmd
from concourse.tile_rust import add_dep_helper

F32 = mybir.dt.float32
BF16 = mybir.dt.bfloat16
F8 = mybir.dt.float8e4
AF = mybir.ActivationFunctionType

N_CORES = 8
B, T, C, H = 4, 2048, 1024, 16
HD = C // H            # 64 head dim
HPC = H // N_CORES     # 2 heads per core
FPC = HPC * HD         # 128 features per core
BT = B * T             # 8192 rows
TCHUNK = 512           # t chunk in qkv phase
NT_CHUNKS = BT // TCHUNK
QB = 512               # query block
NQB = T // QB          # 4 per batch
JTN = T // 128         # 16 j-tiles per batch
ROWS = BT // N_CORES   # 1024 rows per core after AllToAll
KC = C // 128          # 8 contraction tiles over C
VW = 80                # [V | 1 | pad] row unit in vsb
STRIPE = ROWS // B     # 256 rows per (core, batch): striped AllToAll
SCALE = 1.0 / np.sqrt(HD)

# If the overlapped (per-batch, concurrent-with-compute) AllToAll turns
# out to corrupt data on HW, set False to emit all collectives after the
# attention loop (still chunked and pipelined with the out projection).
OVERLAP_A2A = True

# AllToAll chunking: chunk i ships CHUNK_ROWS[i] rows per (core, dest).
# Chunks 0-2 are batches 0-2 (dest r takes that batch's contiguous rows
# r*256..+256 -> one contiguous stage DMA per dest+head, clustered per
# batch: scattered small staging DMAs during attention deepen HW power
# throttling). Batch 3 fires as qb0+qb1 (after qb1), qb2, and qb3 so the
# collective gating the tail is only 128KB.
CHUNK_ROWS = (256, 256, 256, 128, 64, 64)
CHUNK_R0 = (0, 256, 512, 768, 896, 960)      # device-row base per chunk
CHUNK_GBASE = (0, 2048, 4096, 6144, 7168, 7680)  # global-row base per chunk
N_CHUNKS = len(CHUNK_ROWS)

# Interleave qkv/proj matmuls between attention pairs. Keeps every engine
# busy, but the extra concurrency deepens the chip's activity-based power
# throttle and measured net-slower on HW; False = phase-separated.
INTERLEAVE = False

LAST_RESULTS = None    # test.py reads exec_time_ns off this


def build_program(nc):
    xT = nc.dram_tensor("xT", [C, BT], BF16, kind="ExternalInput").ap()
    wq = nc.dram_tensor("wq", [C, FPC], BF16, kind="ExternalInput").ap()
    wk = nc.dram_tensor("wk", [C, FPC], BF16, kind="ExternalInput").ap()
    wv = nc.dram_tensor("wv", [C, FPC], BF16, kind="ExternalInput").ap()
    bqkv = nc.dram_tensor("bqkv", [3, FPC], F32, kind="ExternalInput").ap()
    wp = nc.dram_tensor("wp", [C, C], BF16, kind="ExternalInput").ap()
    bp = nc.dram_tensor("bp", [C], F32, kind="ExternalInput").ap()
    msk = nc.dram_tensor("msk", [128, 128], BF16, kind="ExternalInput").ap()
    out = nc.dram_tensor("out", [ROWS, C], F32, kind="ExternalOutput").ap()
    # round-robin 128-row blocks: global block m (of 64) -> core m%8, so a
    # collective chunk covering any 8*L consecutive blocks addresses all 8
    # destinations. Chunks fire after b1, b2, (b3,qb1), (b3,qb3) with
    # L = 4, 2, 1, 1 blocks per destination.
    cc_in = [
        nc.dram_tensor(f"cc_in{i}", [N_CORES, FPC, R], BF16, kind="Internal").ap()
        for i, R in enumerate(CHUNK_ROWS)
    ]
    cc_out = [
        nc.dram_tensor(f"cc_out{i}", [N_CORES, FPC, R], BF16, kind="Internal").ap()
        for i, R in enumerate(CHUNK_ROWS)
    ]

    with tile.TileContext(nc) as tc:
        with ExitStack() as ctx:
            emit(ctx, tc, xT, wq, wk, wv, bqkv, wp, bp, msk, out, cc_in, cc_out)
    return nc


def emit(ctx, tc, xT, wq, wk, wv, bqkv, wp, bp, msk, out, cc_in, cc_out):
    nc = tc.nc
    res = ctx.enter_context(tc.tile_pool(name="resident", bufs=1))

    # ---------- resident SBUF ----------
    qT = res.tile([128, BT], BF16)
    kT = res.tile([128, BT], BF16)
    vsb = res.tile([128, B * JTN, HPC, VW], BF16)     # [V | 1 | pad] per j-tile/head
    wq_sb = res.tile([128, KC, FPC], BF16)
    wk_sb = res.tile([128, KC, FPC], BF16)
    wv_sb = res.tile([128, KC, FPC], BF16)
    b_sb = res.tile([128, 3], F32)
    wp_sb = res.tile([128, KC, C], BF16)
    bp_sb = res.tile([128, C], F32)
    yT0 = res.tile([64, BT], BF16)
    yT1 = res.tile([64, BT], BF16)
    msk_sb = res.tile([128, 128], BF16)
    yfull = [res.tile([128, KC, R], BF16, name=f"yf{i}")
             for i, R in enumerate(CHUNK_ROWS)]

    # ---------- constant/weight loads (wp/bp deferred to phase 3) ----------
    nc.sync.dma_start(wq_sb[:], wq.rearrange("(a p) m -> p a m", p=128))
    nc.sync.dma_start(wk_sb[:], wk.rearrange("(a p) m -> p a m", p=128))
    nc.sync.dma_start(wv_sb[:], wv.rearrange("(a p) m -> p a m", p=128))
    nc.sync.dma_start(b_sb[:], bqkv.rearrange("b p -> p b"))
    nc.sync.dma_start(msk_sb[:], msk)
    nc.vector.memset(vsb[:, :, :, HD : HD + 1], 1.0)

    # ---------- pools ----------
    # PSUM is 8 banks. INTERLEAVE shares one 2-bank tag across qkv+proj so
    # everything coexists; phase-separated mode scopes pools per phase for
    # deeper per-phase buffering (qkv 4 banks -> scores 6+yps 2 -> proj 4).
    xpool = ctx.enter_context(tc.tile_pool(name="xt", bufs=3))
    ptpool = ctx.enter_context(tc.tile_pool(name="pt", bufs=3))
    npool = ctx.enter_context(tc.tile_pool(name="norm", bufs=3))
    ospool = ctx.enter_context(tc.tile_pool(name="osb", bufs=3))
    ypool = ctx.enter_context(tc.tile_pool(name="yps", bufs=2, space="PSUM"))
    pools = {}
    ph1 = ExitStack()
    if INTERLEAVE:
        qkvps = ctx.enter_context(tc.tile_pool(name="qkvps", bufs=2, space="PSUM"))
        spool = ctx.enter_context(tc.tile_pool(name="sps", bufs=2, space="PSUM"))
        pools["qkv"] = pools["v"] = pools["proj"] = qkvps
        pools["qtag"] = pools["vtag"] = pools["ptag0"] = pools["ptag1"] = "qkvps"
    else:
        pools["qkv"] = ph1.enter_context(tc.tile_pool(name="qkvps", bufs=2, space="PSUM"))
        pools["v"] = ph1.enter_context(tc.tile_pool(name="vps", bufs=2, space="PSUM"))
        pools["qtag"], pools["vtag"] = "qkvps", "vps"
        pools["ptag0"], pools["ptag1"] = "ops0", "ops1"
    xT_t = xT.rearrange("(a p) t -> p a t", p=128)
    NSUB = TCHUNK // 128
    yT = (yT0, yT1)

    # ---------- filler queue: qkv/proj PE work interleaved between pairs ----
    # Attention is ScalarE(exp)-paced; the PE would idle ~0.5us per pair.
    # Instead of monolithic phases, qkv-projection and output-projection
    # matmuls are queued as small closures and popped between attention
    # pairs, keeping both engines saturated from ~25us onward.
    filler = deque()

    def pop_filler(n):
        done = 0
        while done < n and filler:
            f = filler.popleft()
            if callable(f):
                f()
                done += 1

    def drain_filler(marker):
        while marker in filler:
            f = filler.popleft()
            if f is not marker:
                f()

    def drain_all_filler():
        while filler:
            f = filler.popleft()
            if callable(f):
                f()

    chunk_marker = {}

    def reg_chunk(tci):
        t0 = tci * TCHUNK
        xt = xpool.tile([128, KC, TCHUNK], BF16, tag="xt", name=f"xt{tci}")
        # split the 1 MiB chunk load across DMA queues, issued now (one
        # qb-block of lead time before the matmul closures drain); chunk 0
        # gates the first matmul, so split it per a-tile
        nspl = 8 if tci == 0 else 4
        w = KC // nspl
        for spl in range(nspl):
            nc.sync.dma_start(
                xt[:, w * spl : w * (spl + 1), :],
                xT_t[:, w * spl : w * (spl + 1), t0 : t0 + TCHUNK],
            )
        cell = {}

        def qk_mms(bi, w_sb, a0):
            def go():
                if a0 == 0:
                    cell[bi] = pools["qkv"].tile([128, NSUB, HPC, HD], F32,
                                          tag=pools["qtag"], name=f"qk{tci}_{bi}")
                for a in (a0, a0 + 1):
                    nc.tensor.matmul(
                        cell[bi][:], lhsT=w_sb[:, a, :], rhs=xt[:, a, :],
                        start=(a == 0), stop=(a == KC - 1),
                    )
            return go

        def qk_evict(bi, dst):
            def go():
                # evictions on DVE: keeps ScalarE exp-only.  b_q applied;
                # b_k is softmax-invariant (constant per query row), dropped.
                if bi == 0:
                    nc.vector.tensor_scalar_add(
                        dst[:, t0 : t0 + TCHUNK], cell[bi][:], b_sb[:, 0:1]
                    )
                else:
                    nc.vector.tensor_copy(dst[:, t0 : t0 + TCHUNK], cell[bi][:])
            return go

        def v_mms(tt, a0):
            def go():
                # v in natural layout: out partitions = tokens, lhsT = x^T
                if tt == 0 and a0 == 0:
                    cell[2] = pools["v"].tile([128, NSUB, HPC, HD], F32,
                                         tag=pools["vtag"], name=f"v{tci}")
                for a in range(a0, a0 + 4):
                    nc.tensor.matmul(
                        cell[2][:, tt, :, :],
                        lhsT=xt[:, a, tt * 128 : (tt + 1) * 128],
                        rhs=wv_sb[:, a, :],
                        start=(a == 0), stop=(a == KC - 1),
                    )
            return go

        def v_evict():
            # b_v is folded into b_proj on the host (attn rows sum to 1)
            g0 = NSUB * tci
            nc.vector.tensor_copy(vsb[:, g0 : g0 + NSUB, :, 0:HD], cell[2][:])

        for bi, w_sb, dst in ((0, wq_sb, qT), (1, wk_sb, kT)):
            for a0 in range(0, KC, 2):
                filler.append(qk_mms(bi, w_sb, a0))
            filler.append(qk_evict(bi, dst))
        for tt in range(NSUB):
            for a0 in range(0, KC, 4):
                filler.append(v_mms(tt, a0))
        filler.append(v_evict)
        m = object()
        chunk_marker[tci] = m
        filler.append(m)

    def reg_proj(i):
        # readback emitted here (not at fire time): a queued DMA waiting on
        # a collective would block later staging DMAs behind it in FIFO
        nc.sync.dma_start(yfull[i][:], cc_out[i].rearrange("r p t -> p r t"))
        for tt in range(CHUNK_ROWS[i] // 128 or 1):
            rows = min(128, CHUNK_ROWS[i])
            cell = {}

            def p_mms(tt, cell, a0, rows):
                def go():
                    if a0 == 0:
                        cell[0] = pools["proj"].tile([128, 512], F32,
                                             tag=pools["ptag0"], name=f"op{i}_{tt}_0")
                        cell[1] = pools["proj"].tile([128, 512], F32,
                                             tag=pools["ptag1"], name=f"op{i}_{tt}_1")
                    for a in (a0, a0 + 1):
                        lhsT = yfull[i][:, a, tt * 128 : tt * 128 + rows]
                        nc.tensor.matmul(cell[0][0:rows, 0:512], lhsT=lhsT,
                                         rhs=wp_sb[:, a, 0:512],
                                         start=(a == 0), stop=(a == KC - 1))
                        nc.tensor.matmul(cell[1][0:rows, 0:512], lhsT=lhsT,
                                         rhs=wp_sb[:, a, 512:C],
                                         start=(a == 0), stop=(a == KC - 1))
                return go

            def p_evict(tt, cell, rows):
                def go():
                    osb = ospool.tile([128, C], F32, tag="osb",
                                      name=f"osb{i}_{tt}")
                    nc.vector.tensor_add(osb[0:rows, 0:512],
                                         cell[0][0:rows, 0:512],
                                         bp_sb[0:rows, 0:512])
                    nc.vector.tensor_add(osb[0:rows, 512:C],
                                         cell[1][0:rows, 0:512],
                                         bp_sb[0:rows, 512:C])
                    r0 = CHUNK_R0[i] + tt * 128
                    nc.sync.dma_start(out[r0 : r0 + rows, :],
                                      osb[0:rows, :])
                return go

            for a0 in range(0, KC, 2):
                filler.append(p_mms(tt, cell, a0, rows))
            filler.append(p_evict(tt, cell, rows))

    def stage_batch(b):
        # chunk b: dest r gets the batch's contiguous rows r*256..+256
        for r in range(N_CORES):
            c0 = b * T + r * 2 * 128
            nc.sync.dma_start(cc_in[b][r, 0:HD, :], yT0[:, c0 : c0 + 256])
            nc.sync.dma_start(cc_in[b][r, HD:FPC, :], yT1[:, c0 : c0 + 256])

    def stage_b3_qb(qb):
        # chunk 3 (qb0+qb1, dest r <- b3 row block 128r) or chunk 4/5
        # (qb2/qb3, dest r <- 64-row block within the qb)
        if qb < 2:
            for u in range(4):
                r = 4 * qb + u
                c0 = 3 * T + r * 128
                nc.sync.dma_start(cc_in[3][r, 0:HD, :], yT0[:, c0 : c0 + 128])
                nc.sync.dma_start(cc_in[3][r, HD:FPC, :], yT1[:, c0 : c0 + 128])
        else:
            i = qb + 2
            for r in range(N_CORES):
                c0 = 3 * T + qb * QB + r * 64
                nc.sync.dma_start(cc_in[i][r, 0:HD, :], yT0[:, c0 : c0 + 64])
                nc.sync.dma_start(cc_in[i][r, HD:FPC, :], yT1[:, c0 : c0 + 64])

    last_aff = [None]

    def fire_a2a(i, after_attn=True):
        cc = nc.gpsimd.collective_compute(
            "AllToAll", mybir.AluOpType.bypass,
            ins=[cc_in[i]], outs=[cc_out[i]],
            replica_groups=[list(range(N_CORES))],
        )
        # pin the trigger behind already-emitted attention gpsimd work:
        # the scheduler otherwise hoists its staging-semaphore waits to
        # the front of the gpsimd queue, stalling affine_selects (and with
        # them the whole exp->PV pipeline) for 10-17us per batch boundary
        if after_attn and last_aff[0] is not None:
            add_dep_helper(cc.ins, last_aff[0].ins, True,
                           "defer A2A trigger behind attention")

    # ---------- interleaved qkv + attention ----------
    # Chunk tci=4b+qb+1 registers (DMA now, matmul closures queued) at the
    # start of qb block (b,qb) and is force-drained by that block's end --
    # exactly when block (b,qb+1) first needs its q/k/v.
    reg_chunk(0)
    drain_filler(chunk_marker[0])
    if not INTERLEAVE:
        for tci in range(1, NT_CHUNKS):
            reg_chunk(tci)
        drain_all_filler()
        ph1.close()  # free qkv PSUM banks for the attention pools
        ph23 = ExitStack()
        spool = ph23.enter_context(tc.tile_pool(name="sps", bufs=3, space="PSUM"))
    for b in range(B):
        if b == 1:
            nc.sync.dma_start(wp_sb[:], wp.rearrange("(a p) e -> p a e", p=128))
            bp_bcast = bass.AP(tensor=bp.tensor, offset=bp.offset,
                               ap=[[0, 128], [1, C]])
            nc.sync.dma_start(bp_sb[:], bp_bcast)
        for qb in range(NQB):
            nxt = 4 * b + qb + 1
            if INTERLEAVE and nxt < NT_CHUNKS:
                reg_chunk(nxt)
            if OVERLAP_A2A:
                # output projection for landed A2A chunks becomes filler too
                if b == 2 and qb == 0:
                    reg_proj(0)
                elif b == 3 and qb == 0:
                    reg_proj(1)
                elif b == 3 and qb == 2:
                    reg_proj(2)
                elif b == 3 and qb == 3:
                    reg_proj(3)
                    # chunk 4 (qb2's rows) staged last block; firing here
                    # overlaps its collective with qb3's attention
                    fire_a2a(4)
            q0g = b * T + qb * QB
            njt = (qb + 1) * (QB // 128)
            yps = [
                ypool.tile([HD + 1, QB], F32, tag="yps", name=f"yp{b}_{qb}_{h}")
                for h in range(HPC)
            ]
            for pj in range(njt // 2):
                i0s = []
                sps = [spool.tile([128, 2 * QB], F32, tag="sps", name=f"sp{b}_{qb}_{pj}_{h}")
                       for h in range(HPC)]
                pts = [ptpool.tile([128, 2 * QB], BF16, tag="pt", name=f"pt{b}_{qb}_{pj}_{h}")
                       for h in range(HPC)]
                for jj in range(2):
                    j = 2 * pj + jj
                    j0g = b * T + j * 128
                    i0 = max(0, j * 128 - qb * QB)  # first unmasked query col
                    i0s.append(i0)
                    for h in range(HPC):
                        hs = slice(h * HD, (h + 1) * HD)
                        nc.tensor.matmul(
                            sps[h][:, jj * QB + i0 : (jj + 1) * QB],
                            lhsT=kT[hs, j0g : j0g + 128],
                            rhs=qT[hs, q0g + i0 : q0g + QB], start=True, stop=True,
                        )
                for h in range(HPC):
                    # one exp per j-tile pair; [512+i0s[0] : 512+i0s[1]) holds
                    # exp(garbage) but is never read (PV skips masked cols)
                    nc.scalar.activation(
                        pts[h][:, i0s[0] : 2 * QB], sps[h][:, i0s[0] : 2 * QB],
                        AF.Exp, scale=float(SCALE),
                    )
                    for jj in range(2):
                        j = 2 * pj + jj
                        if j * 128 + 127 > qb * QB:
                            # boundary tile: keep where j <= i on the 128-wide band
                            i0 = i0s[jj]
                            band = slice(jj * QB + i0, jj * QB + i0 + 128)
                            last_aff[0] = nc.gpsimd.affine_select(
                                pts[h][:, band], pts[h][:, band],
                                pattern=[[1, 128]], base=0, channel_multiplier=-1,
                                compare_op=mybir.AluOpType.is_ge, fill=0.0,
                            )
                for jj in range(2):
                    j = 2 * pj + jj
                    i0 = i0s[jj]
                    for h in range(HPC):
                        nc.tensor.matmul(
                            yps[h][:, i0:QB],
                            lhsT=vsb[:, b * JTN + j, h, 0 : HD + 1],
                            rhs=pts[h][:, jj * QB + i0 : (jj + 1) * QB],
                            start=(j == 0), stop=(j == njt - 1),
                        )
                if INTERLEAVE:
                    pop_filler(4)
            # softmax normalization: row HD of yp is the denominator.
            # One fast PSUM->SBUF copy releases the yp bank; the recip /
            # broadcast / scale chain then runs off SBUF.
            for h in range(HPC):
                ln = npool.tile([1, QB], F32, tag="ln")
                nc.vector.tensor_copy(ln[:], yps[h][HD : HD + 1, :])
                yraw = npool.tile([HD, QB], F32, tag="yraw")
                nc.vector.tensor_copy(yraw[:], yps[h][0:HD, :])
                rn = npool.tile([1, QB], F32, tag="rn")
                nc.vector.reciprocal_approx_fast(rn[:], ln[:])
                rb = npool.tile([HD, QB], F32, tag="rb")
                nc.gpsimd.partition_broadcast(rb[:], rn[:], channels=HD)
                nc.vector.tensor_mul(yT[h][:, q0g : q0g + QB], yraw[:], rb[:])
            if INTERLEAVE and nxt < NT_CHUNKS:
                drain_filler(chunk_marker[nxt])
            if OVERLAP_A2A:
                # stage at the producing qb's end; defer each fire by one
                # qb-block so the trigger's semaphore waits (which block the
                # gpsimd queue, and with it the next affine_selects) resolve
                # before the trigger reaches the queue head.
                if qb == 1 and b > 0:
                    # two qb-blocks after staging: the trigger's semaphore
                    # waits are fully resolved by now (staging takes
                    # 10-17us), so the gpsimd queue never stalls on them
                    fire_a2a(b - 1)
                if b < 3 and qb == 3:
                    stage_batch(b)
                elif b == 3:
                    if qb == 2:
                        fire_a2a(3)
                    stage_b3_qb(qb)
                    if qb == 3:
                        fire_a2a(5, after_attn=False)

    if not INTERLEAVE:
        ph23.close()  # free the scores banks; proj gets its own 4 banks
        pools["proj"] = ctx.enter_context(
            tc.tile_pool(name="ops", bufs=2, space="PSUM"))
    drain_all_filler()

    if not OVERLAP_A2A:
        for b in range(3):
            stage_batch(b)
        for qb in range(NQB):
            stage_b3_qb(qb)
        for i in range(N_CHUNKS):
            fire_a2a(i)
        for i in range(4):
            reg_proj(i)

    # tail: projection for the last two (small) A2A chunks
    reg_proj(4)
    reg_proj(5)
    drain_all_filler()


_COMPILED_NC = None


def _get_nc():
    global _COMPILED_NC
    if _COMPILED_NC is None:
        nc = bacc.Bacc("TRN2", target_bir_lowering=False, debug=False,
                       num_devices=N_CORES)
        build_program(nc)
        nc.compile()
        _COMPILED_NC = nc
    return _COMPILED_NC


def kernel(x, W_attn, b_attn, W_proj, b_proj):
    global LAST_RESULTS
    nc = _get_nc()

    bf = ml_dtypes.bfloat16
    xT_np = np.ascontiguousarray(
        np.asarray(x, np.float32).reshape(BT, C).T
    ).astype(bf)
    W_attn = np.asarray(W_attn, np.float32)
    b_attn = np.asarray(b_attn, np.float32)
    W_proj = np.asarray(W_proj, np.float32)
    wp_np = W_proj.astype(bf)
    # b_v folds into b_proj: attention rows sum to 1, so y += b_v exactly
    bp_np = (np.asarray(b_proj, np.float32) + b_attn[2 * C :] @ W_proj).astype(np.float32)
    # causal band mask in S^T layout: keep where query col >= key row
    msk_np = np.triu(np.ones((128, 128), np.float32)).astype(bf)

    in_maps = []
    for c in range(N_CORES):
        s = slice(c * FPC, (c + 1) * FPC)
        in_maps.append({
            "xT": xT_np,
            "wq": np.ascontiguousarray(W_attn[:, s]).astype(bf),
            "wk": np.ascontiguousarray(W_attn[:, C:2 * C][:, s]).astype(bf),
            "wv": np.ascontiguousarray(W_attn[:, 2 * C:][:, s]).astype(bf),
            "bqkv": np.ascontiguousarray(
                np.stack([b_attn[s], b_attn[C:2 * C][s], b_attn[2 * C:][s]])
            ).astype(np.float32),
            "wp": wp_np,
            "bp": bp_np,
            "msk": msk_np,
        })

    res = run_bass_kernel_spmd(nc, in_maps, core_ids=list(range(N_CORES)))
    LAST_RESULTS = res
    # gather: chunk i of core c covers global rows GBASE[i] + c*ROWS[i] ..
    arr = np.stack([res.results[c]["out"] for c in range(N_CORES)], axis=0)
    full = np.empty((BT, C), np.float32)
    for i in range(N_CHUNKS):
        g, r0, rows = CHUNK_GBASE[i], CHUNK_R0[i], CHUNK_ROWS[i]
        for c in range(N_CORES):
            full[g + c * rows : g + (c + 1) * rows] = arr[c, r0 : r0 + rows]
    return full.reshape(B, T, C)
